# revision 11
# baseline (speedup 1.0000x reference)
"""Causal self-attention with KV cache on 8 Trainium2 NeuronCores.

Data-parallel over batch: B=128 -> 16 batches per core.

Math (per batch b):
    qkv = x @ W_qkv ; q,k_new,v_new heads of Dh=32
    k = [past_k ; k_new], v = [past_v ; v_new]   (T_full = 512)
    att = softmax(q k^T / sqrt(Dh) + causal)     (q_i sees keys j <= 256+i)
    out = (att @ v) reshaped -> @ W_proj

On-device layout (all fp32):
    xT   [B, D=128, T=256]          x transposed (host prep)
    pkT  [B, 128, 256]              past_k packed-transposed: row 32h+d, col j
    pv   [B, 256, 128]              past_v packed-natural: row j, col 32h+d
    Per batch the kernel computes, entirely via PE-friendly layouts:
      qT, k_newT [feat=(h,d), tok]  via  W^T-stationary matmuls on xT
      v_new [tok, feat]             via  xT-stationary matmuls
      scores^T chunks [keys, 4h x q] via row-tiled (K=32) matmuls, 4 heads
                                     packed into the 128x128 PE array
      e = exp(scale*s) on ACT, causal triangles zeroed via gpsimd affine_select
      oT[(h,d), q] and denom[(h,d), q] accumulated via col-tiled matmuls
                                     (lhsT = v chunk / ones, col_grp = head)
      o_normT = oT * recip(denom)   (DVE)
      y = o_normT^T @ W_proj        (o_normT is directly the lhsT)
"""

import os
import numpy as np

import concourse.bass as bass
import concourse.bacc as bacc
import concourse.mybir as mybir
import concourse.tile as tile
from concourse.bass_utils import run_bass_kernel_spmd

F32 = mybir.dt.float32
F32R = mybir.dt.float32r
BF16 = mybir.dt.bfloat16
# Score matmuls run in float32r (single-pass PE, ~1e-4 rel err; fp32 runs
# 2 HW passes). The o/denominator accumulation can't use f32r (col-tiled
# dst partitions are rejected), so it is fp32 (exact) or bf16 (fast).
SCORE_DT = F32R if os.environ.get("KB_SCORE_F32R", "1") == "1" else F32
OD_BF16 = os.environ.get("KB_OD_BF16", "0") == "1"
OD_DT = BF16 if OD_BF16 else F32

B_FULL = 128
N_CORES = 8
B_LOC = B_FULL // N_CORES  # 16
T = 256
D = 128
H = 4
DH = 32
T_PAST = 256
T_FULL = T_PAST + T  # 512
SCALE = 1.0 / float(np.sqrt(DH))
N_CHUNKS = 4  # key chunks of 128 over T_FULL


def build_nc(n_batches: int = B_LOC) -> bass.Bass:
    nc = bacc.Bacc("TRN2", target_bir_lowering=False, debug=False)

    xT = nc.dram_tensor("xT", [n_batches, D, T], F32, kind="ExternalInput").ap()
    pkT = nc.dram_tensor("pkT", [n_batches, D, T_PAST], SCORE_DT, kind="ExternalInput").ap()
    pv = nc.dram_tensor("pv", [n_batches, T_PAST, D], OD_DT, kind="ExternalInput").ap()
    wqkv = nc.dram_tensor("wqkv", [D, 3 * D], F32, kind="ExternalInput").ap()
    wproj = nc.dram_tensor("wproj", [D, D], SCORE_DT, kind="ExternalInput").ap()

    y = nc.dram_tensor("y", [n_batches, T, D], F32, kind="ExternalOutput").ap()
    knewT = nc.dram_tensor("knewT", [n_batches, D, T], F32, kind="ExternalOutput").ap()
    vnew = nc.dram_tensor("vnew", [n_batches, T, D], F32, kind="ExternalOutput").ap()

    with tile.TileContext(nc) as tc:
        with (
            tc.tile_pool(name="singles", bufs=1) as singles,
            tc.tile_pool(name="inbuf", bufs=4) as inbuf,
            tc.tile_pool(name="proj", bufs=4) as proj,
            tc.tile_pool(name="ebuf", bufs=4) as ebuf,
            tc.tile_pool(name="obuf", bufs=3) as obuf,
            tc.tile_pool(name="ps_e", bufs=1, space="PSUM") as ps_e,
            tc.tile_pool(name="ps_od", bufs=2, space="PSUM") as ps_od,
        ):
            sb_wqkv = singles.tile([D, 3 * D], F32)
            nc.sync.dma_start(out=sb_wqkv, in_=wqkv)
            sb_wproj = singles.tile([D, D], SCORE_DT)
            nc.sync.dma_start(out=sb_wproj, in_=wproj)
            sb_ones = singles.tile([D, DH], OD_DT)
            nc.vector.memset(sb_ones, 1.0)
            # HAM heater: tiled/f32r matmuls don't register as PE activity,
            # so the clock gate keeps the array at 1.2 GHz. A periodic plain
            # bf16 LDWEIGHTS feeds the activity monitor (no PSUM touched).
            sb_heat = singles.tile([D, 2], BF16)
            nc.vector.memset(sb_heat, 1.0)
            # scores PSUM: one bank per head (concurrent row-tiled matmuls
            # sharing a bank hard-fault the device); column halves parity-
            # cycle across chunks for double buffering. Matmuls on the same
            # row strip never overlap, so the two halves of a bank are safe.
            e_ps = ps_e.tile([128, H, 512], F32, tag="e")

            for b in range(n_batches):
                # ---- P1: load + projections (fp32: k/v outputs stay exact)
                sb_xT = inbuf.tile([D, T], F32, tag="xT")
                nc.sync.dma_start(out=sb_xT, in_=xT[b])
                sb_pkT = inbuf.tile([D, T_PAST], SCORE_DT, tag="pkT")
                nc.sync.dma_start(out=sb_pkT, in_=pkT[b])
                sb_pv = inbuf.tile([128, 2, D], OD_DT, tag="pv")
                nc.sync.dma_start(
                    out=sb_pv, in_=pv[b].rearrange("(c p) f -> p c f", p=128)
                )

                qk_ps = ps_od.tile([128, 2, T], F32, tag="o")
                nc.tensor.matmul(
                    out=qk_ps[:, 0, :], lhsT=sb_wqkv[:, 0:D], rhs=sb_xT,
                    start=True, stop=True,
                )
                nc.tensor.matmul(
                    out=qk_ps[:, 1, :], lhsT=sb_wqkv[:, D : 2 * D], rhs=sb_xT,
                    start=True, stop=True,
                )
                v_ps = ps_od.tile([128, 2, D], F32, tag="d")
                for half in range(2):
                    nc.tensor.matmul(
                        out=v_ps[:, half, :],
                        lhsT=sb_xT[:, half * 128 : (half + 1) * 128],
                        rhs=sb_wqkv[:, 2 * D : 3 * D],
                        start=True, stop=True,
                    )

                # compute-dtype copies (bit-identical for f32r)
                sb_qT = proj.tile([D, T], SCORE_DT, tag="qT")
                nc.vector.tensor_copy(sb_qT, qk_ps[:, 0, :])
                sb_kT = proj.tile([D, T], SCORE_DT, tag="kT")
                nc.vector.tensor_copy(sb_kT, qk_ps[:, 1, :])
                sb_kTo = proj.tile([D, T], F32, tag="kTo")
                nc.vector.tensor_copy(sb_kTo, qk_ps[:, 1, :])
                sb_v = proj.tile([128, 2, D], F32, tag="v")
                nc.vector.tensor_copy(sb_v, v_ps)
                if OD_BF16:
                    sb_vb = proj.tile([128, 2, D], OD_DT, tag="vb")
                    nc.vector.tensor_copy(sb_vb, v_ps)
                else:
                    sb_vb = sb_v

                nc.sync.dma_start(out=knewT[b], in_=sb_kTo)
                nc.sync.dma_start(
                    out=vnew[b].rearrange("(c p) f -> p c f", p=128), in_=sb_v
                )

                # ---- P2: attention ---------------------------------------
                # oT[(h,d), q] and denom accumulators in separate banks: a
                # start=True matmul marks its partitions' whole 2KB zero
                # region, so the two groups cannot share a bank.
                o_ps = ps_od.tile([128, T], F32, tag="o")
                d_ps = ps_od.tile([128, T], F32, tag="d")

                for c in range(N_CHUNKS):
                    if c < 2:
                        ksrc = sb_pkT[:, c * 128 : (c + 1) * 128]
                    else:
                        ksrc = sb_kT[:, (c - 2) * 128 : (c - 1) * 128]
                    vsrc = sb_pv[:, c, :] if c < 2 else sb_vb[:, c - 2, :]

                    # chunk 3 keys (384..511) only visible to queries >= 128
                    q0 = 128 if c == 3 else 0
                    nq = T - q0

                    par = (b * N_CHUNKS + c) % 2
                    ecols = slice(par * 256, par * 256 + nq)
                    for h in range(H):
                        nc.tensor.matmul(
                            out=e_ps[:, h, ecols],
                            lhsT=ksrc[32 * h : 32 * h + 32, :],
                            rhs=sb_qT[32 * h : 32 * h + 32, q0:T],
                            start=True, stop=True,
                            tile_position=(32 * h, 0),
                        )

                    nc.tensor.ldweights(weights=sb_heat)
                    sb_e = ebuf.tile([128, H, nq], OD_DT, tag="e")
                    nc.scalar.activation(
                        out=sb_e, in_=e_ps[:, :, ecols],
                        func=mybir.ActivationFunctionType.Exp, scale=SCALE,
                    )

                    if c >= 2:
                        # zero the causal triangle: keep iff q_col >= key_row
                        nc.gpsimd.affine_select(
                            out=sb_e[:, :, 0:128], in_=sb_e[:, :, 0:128],
                            compare_op=mybir.AluOpType.is_ge,
                            fill=0.0, base=0,
                            pattern=[[0, H], [1, 128]],
                            channel_multiplier=-1,
                        )

                    # accumulate oT and denom via col-tiled matmuls; query
                    # cols 0:128 finish at c==2, cols 128:256 at c==3.
                    for h in range(H):
                        hs = slice(32 * h, 32 * h + 32)
                        tp = (0, 32 * h)
                        qsl = slice(q0, T)
                        nc.tensor.matmul(
                            out=o_ps[hs, qsl], lhsT=vsrc[:, hs],
                            rhs=sb_e[:, h, :],
                            start=(c == 0), stop=(c == 3), tile_position=tp,
                            skip_group_check=True,
                        )
                        nc.tensor.matmul(
                            out=d_ps[hs, qsl], lhsT=sb_ones,
                            rhs=sb_e[:, h, :],
                            start=(c == 0), stop=(c == 3), tile_position=tp,
                            skip_group_check=True,
                        )

                sb_recip = obuf.tile([128, T], F32, tag="recip")
                sb_oT = obuf.tile([128, T], SCORE_DT, tag="oT")
                y_ps = ps_od.tile([128, 2, D], F32, tag="d")
                for half in range(2):
                    hsl = slice(half * 128, (half + 1) * 128)
                    nc.vector.reciprocal(sb_recip[:, hsl], d_ps[:, hsl])
                    nc.vector.tensor_mul(sb_oT[:, hsl], o_ps[:, hsl], sb_recip[:, hsl])
                    nc.tensor.matmul(
                        out=y_ps[:, half, :],
                        lhsT=sb_oT[:, hsl],
                        rhs=sb_wproj,
                        start=True, stop=True,
                    )
                sb_y = obuf.tile([128, 2, D], F32, tag="y")
                nc.vector.tensor_copy(sb_y, y_ps)
                nc.sync.dma_start(
                    out=y[b].rearrange("(c p) f -> p c f", p=128), in_=sb_y
                )

    nc.compile()
    return nc


def _host_prep(x, past_k, past_v):
    import ml_dtypes

    xT = np.ascontiguousarray(x.transpose(0, 2, 1))  # [B, D, T]
    pkT = np.ascontiguousarray(
        past_k.transpose(0, 1, 3, 2).reshape(B_FULL, D, T_PAST)
    )
    pv = np.ascontiguousarray(
        past_v.transpose(0, 2, 1, 3).reshape(B_FULL, T_PAST, D)
    )
    if OD_BF16:
        pv = pv.astype(ml_dtypes.bfloat16)
    return xT, pkT, pv


def run(x, past_k, past_v, W_qkv, W_proj, trace=False, nc=None):
    x = np.asarray(x, dtype=np.float32)
    past_k = np.asarray(past_k, dtype=np.float32)
    past_v = np.asarray(past_v, dtype=np.float32)
    W_qkv = np.ascontiguousarray(np.asarray(W_qkv, dtype=np.float32))
    W_proj = np.ascontiguousarray(np.asarray(W_proj, dtype=np.float32))

    xT, pkT, pv = _host_prep(x, past_k, past_v)

    if nc is None:
        nc = build_nc(B_LOC)
    in_maps = []
    for i in range(N_CORES):
        sl = slice(i * B_LOC, (i + 1) * B_LOC)
        in_maps.append(
            {
                "xT": np.ascontiguousarray(xT[sl]),
                "pkT": np.ascontiguousarray(pkT[sl]),
                "pv": np.ascontiguousarray(pv[sl]),
                "wqkv": W_qkv,
                "wproj": W_proj,
            }
        )

    res = run_bass_kernel_spmd(
        nc, in_maps, core_ids=list(range(N_CORES)), trace=trace
    )

    y = np.concatenate([r["y"] for r in res.results], axis=0)
    knewT = np.concatenate([r["knewT"] for r in res.results], axis=0)
    vnew = np.concatenate([r["vnew"] for r in res.results], axis=0)

    k_new = knewT.reshape(B_FULL, H, DH, T).transpose(0, 1, 3, 2)
    v_new = vnew.reshape(B_FULL, T, H, DH).transpose(0, 2, 1, 3)
    k_full = np.concatenate([past_k, k_new], axis=2)
    v_full = np.concatenate([past_v, v_new], axis=2)
    return (
        np.ascontiguousarray(y),
        np.ascontiguousarray(k_full),
        np.ascontiguousarray(v_full),
    ), res


def kernel(x, past_k, past_v, W_qkv, W_proj):
    out, _ = run(x, past_k, past_v, W_qkv, W_proj, trace=False)
    return out


# revision 12
# speedup vs baseline: 1.1283x; 1.1283x over previous
"""Causal self-attention with KV cache on 8 Trainium2 NeuronCores.

Data-parallel over batch: B=128 -> 16 batches per core.

Math (per batch b):
    qkv = x @ W_qkv ; q,k_new,v_new heads of Dh=32
    k = [past_k ; k_new], v = [past_v ; v_new]   (T_full = 512)
    att = softmax(q k^T / sqrt(Dh) + causal)     (q_i sees keys j <= 256+i)
    out = (att @ v) reshaped -> @ W_proj

On-device layout (all fp32):
    xT   [B, D=128, T=256]          x transposed (host prep)
    pkT  [B, 128, 256]              past_k packed-transposed: row 32h+d, col j
    pv   [B, 256, 128]              past_v packed-natural: row j, col 32h+d
    Per batch the kernel computes, entirely via PE-friendly layouts:
      qT, k_newT [feat=(h,d), tok]  via  W^T-stationary matmuls on xT
      v_new [tok, feat]             via  xT-stationary matmuls
      scores^T chunks [keys, 4h x q] via row-tiled (K=32) matmuls, 4 heads
                                     packed into the 128x128 PE array
      e = exp(scale*s) on ACT, causal triangles zeroed via gpsimd affine_select
      oT[(h,d), q] and denom[(h,d), q] accumulated via col-tiled matmuls
                                     (lhsT = v chunk / ones, col_grp = head)
      o_normT = oT * recip(denom)   (DVE)
      y = o_normT^T @ W_proj        (o_normT is directly the lhsT)
"""

import os
import numpy as np

import concourse.bass as bass
import concourse.bacc as bacc
import concourse.mybir as mybir
import concourse.tile as tile
from concourse.bass_utils import run_bass_kernel_spmd

F32 = mybir.dt.float32
F32R = mybir.dt.float32r
BF16 = mybir.dt.bfloat16
# Score matmuls run in float32r (single-pass PE, ~1e-4 rel err; fp32 runs
# 2 HW passes). The o/denominator accumulation can't use f32r (col-tiled
# dst partitions are rejected), so it is fp32 (exact) or bf16 (fast).
SCORE_DT = F32R if os.environ.get("KB_SCORE_F32R", "1") == "1" else F32
OD_BF16 = os.environ.get("KB_OD_BF16", "0") == "1"
OD_DT = BF16 if OD_BF16 else F32

B_FULL = 128
N_CORES = 8
B_LOC = B_FULL // N_CORES  # 16
T = 256
D = 128
H = 4
DH = 32
T_PAST = 256
T_FULL = T_PAST + T  # 512
SCALE = 1.0 / float(np.sqrt(DH))
N_CHUNKS = 4  # key chunks of 128 over T_FULL


def build_nc(n_batches: int = B_LOC) -> bass.Bass:
    nc = bacc.Bacc("TRN2", target_bir_lowering=False, debug=False)

    xT = nc.dram_tensor("xT", [n_batches, D, T], SCORE_DT, kind="ExternalInput").ap()
    pkT = nc.dram_tensor("pkT", [n_batches, D, T_PAST], SCORE_DT, kind="ExternalInput").ap()
    pv = nc.dram_tensor("pv", [n_batches, T_PAST, D], OD_DT, kind="ExternalInput").ap()
    wqkv = nc.dram_tensor("wqkv", [D, 3 * D], SCORE_DT, kind="ExternalInput").ap()
    wproj = nc.dram_tensor("wproj", [D, D], SCORE_DT, kind="ExternalInput").ap()

    y = nc.dram_tensor("y", [n_batches, T, D], F32, kind="ExternalOutput").ap()
    knewT = nc.dram_tensor("knewT", [n_batches, D, T], SCORE_DT, kind="ExternalOutput").ap()
    vnew = nc.dram_tensor("vnew", [n_batches, T, D], F32, kind="ExternalOutput").ap()

    with tile.TileContext(nc) as tc:
        with (
            tc.tile_pool(name="singles", bufs=1) as singles,
            tc.tile_pool(name="inbuf", bufs=4) as inbuf,
            tc.tile_pool(name="proj", bufs=4) as proj,
            tc.tile_pool(name="ebuf", bufs=4) as ebuf,
            tc.tile_pool(name="obuf", bufs=3) as obuf,
            tc.tile_pool(name="ps_e", bufs=1, space="PSUM") as ps_e,
            tc.tile_pool(name="ps_od", bufs=2, space="PSUM") as ps_od,
        ):
            sb_wqkv = singles.tile([D, 3 * D], SCORE_DT)
            nc.sync.dma_start(out=sb_wqkv, in_=wqkv)
            sb_wproj = singles.tile([D, D], SCORE_DT)
            nc.sync.dma_start(out=sb_wproj, in_=wproj)
            sb_ones = singles.tile([D, DH], OD_DT)
            nc.vector.memset(sb_ones, 1.0)
            # scores PSUM: one bank per head (concurrent row-tiled matmuls
            # sharing a bank hard-fault the device); column halves parity-
            # cycle across chunks for double buffering. Matmuls on the same
            # row strip never overlap, so the two halves of a bank are safe.
            e_ps = ps_e.tile([128, H, 512], F32, tag="e")

            for b in range(n_batches):
                # ---- P1: load + projections (fp32: k/v outputs stay exact)
                sb_xT = inbuf.tile([D, T], SCORE_DT, tag="xT")
                nc.sync.dma_start(out=sb_xT, in_=xT[b])
                sb_pkT = inbuf.tile([D, T_PAST], SCORE_DT, tag="pkT")
                nc.sync.dma_start(out=sb_pkT, in_=pkT[b])
                sb_pv = inbuf.tile([128, 2, D], OD_DT, tag="pv")
                nc.sync.dma_start(
                    out=sb_pv, in_=pv[b].rearrange("(c p) f -> p c f", p=128)
                )

                qk_ps = ps_od.tile([128, 2, T], F32, tag="o")
                nc.tensor.matmul(
                    out=qk_ps[:, 0, :], lhsT=sb_wqkv[:, 0:D], rhs=sb_xT,
                    start=True, stop=True,
                )
                nc.tensor.matmul(
                    out=qk_ps[:, 1, :], lhsT=sb_wqkv[:, D : 2 * D], rhs=sb_xT,
                    start=True, stop=True,
                )
                v_ps = ps_od.tile([128, 2, D], F32, tag="d")
                for half in range(2):
                    nc.tensor.matmul(
                        out=v_ps[:, half, :],
                        lhsT=sb_xT[:, half * 128 : (half + 1) * 128],
                        rhs=sb_wqkv[:, 2 * D : 3 * D],
                        start=True, stop=True,
                    )

                # compute-dtype copies (bit-identical for f32r)
                sb_qT = proj.tile([D, T], SCORE_DT, tag="qT")
                nc.vector.tensor_copy(sb_qT, qk_ps[:, 0, :])
                sb_kT = proj.tile([D, T], SCORE_DT, tag="kT")
                nc.vector.tensor_copy(sb_kT, qk_ps[:, 1, :])

                sb_v = proj.tile([128, 2, D], F32, tag="v")
                nc.vector.tensor_copy(sb_v, v_ps)
                if OD_BF16:
                    sb_vb = proj.tile([128, 2, D], OD_DT, tag="vb")
                    nc.vector.tensor_copy(sb_vb, v_ps)
                else:
                    sb_vb = sb_v

                nc.sync.dma_start(out=knewT[b], in_=sb_kT)
                nc.sync.dma_start(
                    out=vnew[b].rearrange("(c p) f -> p c f", p=128), in_=sb_v
                )

                # ---- P2: attention ---------------------------------------
                # oT[(h,d), q] and denom accumulators in separate banks: a
                # start=True matmul marks its partitions' whole 2KB zero
                # region, so the two groups cannot share a bank.
                o_ps = ps_od.tile([128, T], F32, tag="o")
                d_ps = ps_od.tile([128, T], F32, tag="d")

                for c in range(N_CHUNKS):
                    if c < 2:
                        ksrc = sb_pkT[:, c * 128 : (c + 1) * 128]
                    else:
                        ksrc = sb_kT[:, (c - 2) * 128 : (c - 1) * 128]
                    vsrc = sb_pv[:, c, :] if c < 2 else sb_vb[:, c - 2, :]

                    # chunk 3 keys (384..511) only visible to queries >= 128
                    q0 = 128 if c == 3 else 0
                    nq = T - q0

                    par = (b * N_CHUNKS + c) % 2
                    ecols = slice(par * 256, par * 256 + nq)
                    for h in range(H):
                        nc.tensor.matmul(
                            out=e_ps[:, h, ecols],
                            lhsT=ksrc[32 * h : 32 * h + 32, :],
                            rhs=sb_qT[32 * h : 32 * h + 32, q0:T],
                            start=True, stop=True,
                            tile_position=(32 * h, 0),
                        )

                    sb_e = ebuf.tile([128, H, nq], OD_DT, tag="e")
                    nc.scalar.activation(
                        out=sb_e, in_=e_ps[:, :, ecols],
                        func=mybir.ActivationFunctionType.Exp, scale=SCALE,
                    )

                    if c >= 2:
                        # zero the causal triangle: keep iff q_col >= key_row
                        nc.gpsimd.affine_select(
                            out=sb_e[:, :, 0:128], in_=sb_e[:, :, 0:128],
                            compare_op=mybir.AluOpType.is_ge,
                            fill=0.0, base=0,
                            pattern=[[0, H], [1, 128]],
                            channel_multiplier=-1,
                        )

                    # accumulate oT and denom via col-tiled matmuls; query
                    # cols 0:128 finish at c==2, cols 128:256 at c==3.
                    qsl = slice(q0, T)
                    for h in range(H):
                        nc.tensor.matmul(
                            out=o_ps[32 * h : 32 * h + 32, qsl],
                            lhsT=vsrc[:, 32 * h : 32 * h + 32],
                            rhs=sb_e[:, h, :],
                            start=(c == 0), stop=(c == 3),
                            tile_position=(0, 32 * h),
                            skip_group_check=True,
                        )
                    for h in range(H):
                        nc.tensor.matmul(
                            out=d_ps[32 * h : 32 * h + 32, qsl],
                            lhsT=sb_ones,
                            rhs=sb_e[:, h, :],
                            start=(c == 0), stop=(c == 3),
                            tile_position=(0, 32 * h),
                            skip_group_check=True,
                        )

                sb_recip = obuf.tile([128, T], F32, tag="recip")
                sb_oT = obuf.tile([128, T], SCORE_DT, tag="oT")
                y_ps = ps_od.tile([128, 2, D], F32, tag="d")
                for half in range(2):
                    hsl = slice(half * 128, (half + 1) * 128)
                    nc.vector.reciprocal(sb_recip[:, hsl], d_ps[:, hsl])
                    nc.vector.tensor_mul(sb_oT[:, hsl], o_ps[:, hsl], sb_recip[:, hsl])
                    nc.tensor.matmul(
                        out=y_ps[:, half, :],
                        lhsT=sb_oT[:, hsl],
                        rhs=sb_wproj,
                        start=True, stop=True,
                    )
                sb_y = obuf.tile([128, 2, D], F32, tag="y")
                nc.vector.tensor_copy(sb_y, y_ps)
                nc.sync.dma_start(
                    out=y[b].rearrange("(c p) f -> p c f", p=128), in_=sb_y
                )

    nc.compile()
    return nc


def _host_prep(x, past_k, past_v):
    import ml_dtypes

    xT = np.ascontiguousarray(x.transpose(0, 2, 1))  # [B, D, T]
    pkT = np.ascontiguousarray(
        past_k.transpose(0, 1, 3, 2).reshape(B_FULL, D, T_PAST)
    )
    pv = np.ascontiguousarray(
        past_v.transpose(0, 2, 1, 3).reshape(B_FULL, T_PAST, D)
    )
    if OD_BF16:
        pv = pv.astype(ml_dtypes.bfloat16)
    return xT, pkT, pv


def run(x, past_k, past_v, W_qkv, W_proj, trace=False, nc=None):
    x = np.asarray(x, dtype=np.float32)
    past_k = np.asarray(past_k, dtype=np.float32)
    past_v = np.asarray(past_v, dtype=np.float32)
    W_qkv = np.ascontiguousarray(np.asarray(W_qkv, dtype=np.float32))
    W_proj = np.ascontiguousarray(np.asarray(W_proj, dtype=np.float32))

    xT, pkT, pv = _host_prep(x, past_k, past_v)

    if nc is None:
        nc = build_nc(B_LOC)
    in_maps = []
    for i in range(N_CORES):
        sl = slice(i * B_LOC, (i + 1) * B_LOC)
        in_maps.append(
            {
                "xT": np.ascontiguousarray(xT[sl]),
                "pkT": np.ascontiguousarray(pkT[sl]),
                "pv": np.ascontiguousarray(pv[sl]),
                "wqkv": W_qkv,
                "wproj": W_proj,
            }
        )

    res = run_bass_kernel_spmd(
        nc, in_maps, core_ids=list(range(N_CORES)), trace=trace
    )

    y = np.concatenate([r["y"] for r in res.results], axis=0)
    knewT = np.concatenate([r["knewT"] for r in res.results], axis=0)
    vnew = np.concatenate([r["vnew"] for r in res.results], axis=0)

    k_new = knewT.reshape(B_FULL, H, DH, T).transpose(0, 1, 3, 2)
    v_new = vnew.reshape(B_FULL, T, H, DH).transpose(0, 2, 1, 3)
    k_full = np.concatenate([past_k, k_new], axis=2)
    v_full = np.concatenate([past_v, v_new], axis=2)
    return (
        np.ascontiguousarray(y),
        np.ascontiguousarray(k_full),
        np.ascontiguousarray(v_full),
    ), res


def kernel(x, past_k, past_v, W_qkv, W_proj):
    out, _ = run(x, past_k, past_v, W_qkv, W_proj, trace=False)
    return out


# revision 13
# speedup vs baseline: 1.5967x; 1.4152x over previous
"""Causal self-attention with KV cache on 8 Trainium2 NeuronCores.

Data-parallel over batch: B=128 -> 16 batches per core.

Math (per batch b):
    qkv = x @ W_qkv ; q,k_new,v_new heads of Dh=32
    k = [past_k ; k_new], v = [past_v ; v_new]   (T_full = 512)
    att = softmax(q k^T / sqrt(Dh) + causal)     (q_i sees keys j <= 256+i)
    out = (att @ v) reshaped -> @ W_proj

On-device layout (all fp32):
    xT   [B, D=128, T=256]          x transposed (host prep)
    pkT  [B, 128, 256]              past_k packed-transposed: row 32h+d, col j
    pv   [B, 256, 128]              past_v packed-natural: row j, col 32h+d
    Per batch the kernel computes, entirely via PE-friendly layouts:
      qT, k_newT [feat=(h,d), tok]  via  W^T-stationary matmuls on xT
      v_new [tok, feat]             via  xT-stationary matmuls
      scores^T chunks [keys, 4h x q] via row-tiled (K=32) matmuls, 4 heads
                                     packed into the 128x128 PE array
      e = exp(scale*s) on ACT, causal triangles zeroed via gpsimd affine_select
      oT[(h,d), q] and denom[(h,d), q] accumulated via col-tiled matmuls
                                     (lhsT = v chunk / ones, col_grp = head)
      o_normT = oT * recip(denom)   (DVE)
      y = o_normT^T @ W_proj        (o_normT is directly the lhsT)
"""

import os
import numpy as np

import concourse.bass as bass
import concourse.bacc as bacc
import concourse.mybir as mybir
import concourse.tile as tile
from concourse.bass_utils import run_bass_kernel_spmd

F32 = mybir.dt.float32
F32R = mybir.dt.float32r
BF16 = mybir.dt.bfloat16
# Score matmuls run in float32r (single-pass PE, ~1e-4 rel err; fp32 runs
# 2 HW passes). The o/denominator accumulation can't use f32r (col-tiled
# dst partitions are rejected), so it is fp32 (exact) or bf16 (fast).
SCORE_DT = F32R if os.environ.get("KB_SCORE_F32R", "1") == "1" else F32
OD_BF16 = os.environ.get("KB_OD_BF16", "0") == "1"
OD_DT = BF16 if OD_BF16 else F32

B_FULL = 128
N_CORES = 8
B_LOC = B_FULL // N_CORES  # 16
T = 256
D = 128
H = 4
DH = 32
T_PAST = 256
T_FULL = T_PAST + T  # 512
SCALE = 1.0 / float(np.sqrt(DH))
N_CHUNKS = 4  # key chunks of 128 over T_FULL


def build_nc(n_batches: int = B_LOC) -> bass.Bass:
    nc = bacc.Bacc("TRN2", target_bir_lowering=False, debug=False)

    xT = nc.dram_tensor("xT", [n_batches, D, T], SCORE_DT, kind="ExternalInput").ap()
    pkT = nc.dram_tensor("pkT", [n_batches, D, T_PAST], SCORE_DT, kind="ExternalInput").ap()
    pv = nc.dram_tensor("pv", [n_batches, T_PAST, D], OD_DT, kind="ExternalInput").ap()
    wqkv = nc.dram_tensor("wqkv", [D, 3 * D], SCORE_DT, kind="ExternalInput").ap()
    wproj = nc.dram_tensor("wproj", [D, D], SCORE_DT, kind="ExternalInput").ap()

    y = nc.dram_tensor("y", [n_batches, T, D], F32, kind="ExternalOutput").ap()
    knewT = nc.dram_tensor("knewT", [n_batches, D, T], SCORE_DT, kind="ExternalOutput").ap()
    vnew = nc.dram_tensor("vnew", [n_batches, T, D], F32, kind="ExternalOutput").ap()

    with tile.TileContext(nc) as tc:
        with (
            tc.tile_pool(name="singles", bufs=1) as singles,
            tc.tile_pool(name="inbuf", bufs=4) as inbuf,
            tc.tile_pool(name="proj", bufs=4) as proj,
            tc.tile_pool(name="ebuf", bufs=4) as ebuf,
            tc.tile_pool(name="obuf", bufs=3) as obuf,
            tc.tile_pool(name="ps_e", bufs=1, space="PSUM") as ps_e,
            tc.tile_pool(name="ps_od", bufs=2, space="PSUM") as ps_od,
        ):
            sb_wqkv = singles.tile([D, 3 * D], SCORE_DT)
            nc.sync.dma_start(out=sb_wqkv, in_=wqkv)
            sb_wproj = singles.tile([D, D], SCORE_DT)
            nc.sync.dma_start(out=sb_wproj, in_=wproj)
            sb_ones = singles.tile([D, DH], OD_DT)
            nc.vector.memset(sb_ones, 1.0)
            # scores PSUM: one bank per head (concurrent row-tiled matmuls
            # sharing a bank hard-fault the device); column halves parity-
            # cycle across chunks for double buffering. Matmuls on the same
            # row strip never overlap, so the two halves of a bank are safe.
            e_ps = ps_e.tile([128, H, 512], F32, tag="e")

            def emit_p1(b):
                sb_xT = inbuf.tile([D, T], SCORE_DT, tag="xT")
                nc.sync.dma_start(out=sb_xT, in_=xT[b])
                sb_pkT = inbuf.tile([D, T_PAST], SCORE_DT, tag="pkT")
                nc.sync.dma_start(out=sb_pkT, in_=pkT[b])
                sb_pv = inbuf.tile([128, 2, D], OD_DT, tag="pv")
                nc.sync.dma_start(
                    out=sb_pv, in_=pv[b].rearrange("(c p) f -> p c f", p=128)
                )

                qk_ps = ps_od.tile([128, 2, T], F32, tag="o")
                nc.tensor.matmul(
                    out=qk_ps[:, 0, :], lhsT=sb_wqkv[:, 0:D], rhs=sb_xT,
                    start=True, stop=True,
                )
                nc.tensor.matmul(
                    out=qk_ps[:, 1, :], lhsT=sb_wqkv[:, D : 2 * D], rhs=sb_xT,
                    start=True, stop=True,
                )
                v_ps = ps_od.tile([128, 2, D], F32, tag="d")
                for half in range(2):
                    nc.tensor.matmul(
                        out=v_ps[:, half, :],
                        lhsT=sb_xT[:, half * 128 : (half + 1) * 128],
                        rhs=sb_wqkv[:, 2 * D : 3 * D],
                        start=True, stop=True,
                    )

                sb_qT = proj.tile([D, T], SCORE_DT, tag="qT")
                nc.vector.tensor_copy(sb_qT, qk_ps[:, 0, :])
                sb_kT = proj.tile([D, T], SCORE_DT, tag="kT")
                nc.vector.tensor_copy(sb_kT, qk_ps[:, 1, :])
                sb_v = proj.tile([128, 2, D], F32, tag="v")
                nc.vector.tensor_copy(sb_v, v_ps)
                if OD_BF16:
                    sb_vb = proj.tile([128, 2, D], OD_DT, tag="vb")
                    nc.vector.tensor_copy(sb_vb, v_ps)
                else:
                    sb_vb = sb_v

                nc.sync.dma_start(out=knewT[b], in_=sb_kT)
                nc.sync.dma_start(
                    out=vnew[b].rearrange("(c p) f -> p c f", p=128), in_=sb_v
                )
                o_ps = ps_od.tile([128, T], F32, tag="o")
                d_ps = ps_od.tile([128, T], F32, tag="d")
                return dict(qT=sb_qT, kT=sb_kT, v=sb_v, vb=sb_vb,
                            pkT=sb_pkT, pv=sb_pv, o=o_ps, d=d_ps)

            def emit_scores(b, c, ctx):
                if c < 2:
                    ksrc = ctx["pkT"][:, c * 128 : (c + 1) * 128]
                else:
                    ksrc = ctx["kT"][:, (c - 2) * 128 : (c - 1) * 128]
                q0 = 128 if c == 3 else 0
                nq = T - q0
                par = (b * N_CHUNKS + c) % 2
                ecols = slice(par * 256, par * 256 + nq)
                for h in range(H):
                    nc.tensor.matmul(
                        out=e_ps[:, h, ecols],
                        lhsT=ksrc[32 * h : 32 * h + 32, :],
                        rhs=ctx["qT"][32 * h : 32 * h + 32, q0:T],
                        start=True, stop=True,
                        tile_position=(32 * h, 0),
                    )
                sb_e = ebuf.tile([128, H, nq], OD_DT, tag="e")
                nc.scalar.activation(
                    out=sb_e, in_=e_ps[:, :, ecols],
                    func=mybir.ActivationFunctionType.Exp, scale=SCALE,
                )
                if c >= 2:
                    # zero the causal triangle: keep iff q_col >= key_row
                    nc.gpsimd.affine_select(
                        out=sb_e[:, :, 0:128], in_=sb_e[:, :, 0:128],
                        compare_op=mybir.AluOpType.is_ge,
                        fill=0.0, base=0,
                        pattern=[[0, H], [1, 128]],
                        channel_multiplier=-1,
                    )
                return sb_e

            def emit_od(b, c, ctx, sb_e):
                vsrc = ctx["pv"][:, c, :] if c < 2 else ctx["vb"][:, c - 2, :]
                q0 = 128 if c == 3 else 0
                qsl = slice(q0, T)
                for h in range(H):
                    nc.tensor.matmul(
                        out=ctx["o"][32 * h : 32 * h + 32, qsl],
                        lhsT=vsrc[:, 32 * h : 32 * h + 32],
                        rhs=sb_e[:, h, :],
                        start=(c == 0), stop=(c == 3),
                        tile_position=(0, 32 * h),
                        skip_group_check=True,
                    )
                for h in range(H):
                    nc.tensor.matmul(
                        out=ctx["d"][32 * h : 32 * h + 32, qsl],
                        lhsT=sb_ones,
                        rhs=sb_e[:, h, :],
                        start=(c == 0), stop=(c == 3),
                        tile_position=(0, 32 * h),
                        skip_group_check=True,
                    )

            def emit_div_p3(b, ctx):
                sb_recip = obuf.tile([128, T], F32, tag="recip")
                sb_oT = obuf.tile([128, T], SCORE_DT, tag="oT")
                y_ps = ps_od.tile([128, 2, D], F32, tag="o")
                for half in range(2):
                    hsl = slice(half * 128, (half + 1) * 128)
                    nc.vector.reciprocal(sb_recip[:, hsl], ctx["d"][:, hsl])
                    nc.vector.tensor_mul(
                        sb_oT[:, hsl], ctx["o"][:, hsl], sb_recip[:, hsl]
                    )
                    nc.tensor.matmul(
                        out=y_ps[:, half, :],
                        lhsT=sb_oT[:, hsl],
                        rhs=sb_wproj,
                        start=True, stop=True,
                    )
                sb_y = obuf.tile([128, 2, D], F32, tag="y")
                nc.vector.tensor_copy(sb_y, y_ps)
                nc.sync.dma_start(
                    out=y[b].rearrange("(c p) f -> p c f", p=128), in_=sb_y
                )

            # Software-pipelined emission: the PE executes its queue in
            # order, so o/d matmuls (which wait on exp) are emitted two
            # chunks behind their scores, and each batch's division +
            # output projection is deferred into the next batch.
            odq = []
            divq = []
            for b in range(n_batches):
                ctx = emit_p1(b)
                for c in range(N_CHUNKS):
                    sb_e = emit_scores(b, c, ctx)
                    odq.append((b, c, ctx, sb_e))
                    while len(odq) > 2:
                        nb, nck, nctx, ne = odq.pop(0)
                        emit_od(nb, nck, nctx, ne)
                    if c == 2 and divq:
                        emit_div_p3(*divq.pop(0))
                divq.append((b, ctx))
            while odq:
                nb, nck, nctx, ne = odq.pop(0)
                emit_od(nb, nck, nctx, ne)
            while divq:
                emit_div_p3(*divq.pop(0))

    nc.compile()
    return nc


def _host_prep(x, past_k, past_v):
    import ml_dtypes

    xT = np.ascontiguousarray(x.transpose(0, 2, 1))  # [B, D, T]
    pkT = np.ascontiguousarray(
        past_k.transpose(0, 1, 3, 2).reshape(B_FULL, D, T_PAST)
    )
    pv = np.ascontiguousarray(
        past_v.transpose(0, 2, 1, 3).reshape(B_FULL, T_PAST, D)
    )
    if OD_BF16:
        pv = pv.astype(ml_dtypes.bfloat16)
    return xT, pkT, pv


def run(x, past_k, past_v, W_qkv, W_proj, trace=False, nc=None):
    x = np.asarray(x, dtype=np.float32)
    past_k = np.asarray(past_k, dtype=np.float32)
    past_v = np.asarray(past_v, dtype=np.float32)
    W_qkv = np.ascontiguousarray(np.asarray(W_qkv, dtype=np.float32))
    W_proj = np.ascontiguousarray(np.asarray(W_proj, dtype=np.float32))

    xT, pkT, pv = _host_prep(x, past_k, past_v)

    if nc is None:
        nc = build_nc(B_LOC)
    in_maps = []
    for i in range(N_CORES):
        sl = slice(i * B_LOC, (i + 1) * B_LOC)
        in_maps.append(
            {
                "xT": np.ascontiguousarray(xT[sl]),
                "pkT": np.ascontiguousarray(pkT[sl]),
                "pv": np.ascontiguousarray(pv[sl]),
                "wqkv": W_qkv,
                "wproj": W_proj,
            }
        )

    res = run_bass_kernel_spmd(
        nc, in_maps, core_ids=list(range(N_CORES)), trace=trace
    )

    y = np.concatenate([r["y"] for r in res.results], axis=0)
    knewT = np.concatenate([r["knewT"] for r in res.results], axis=0)
    vnew = np.concatenate([r["vnew"] for r in res.results], axis=0)

    k_new = knewT.reshape(B_FULL, H, DH, T).transpose(0, 1, 3, 2)
    v_new = vnew.reshape(B_FULL, T, H, DH).transpose(0, 2, 1, 3)
    k_full = np.concatenate([past_k, k_new], axis=2)
    v_full = np.concatenate([past_v, v_new], axis=2)
    return (
        np.ascontiguousarray(y),
        np.ascontiguousarray(k_full),
        np.ascontiguousarray(v_full),
    ), res


def kernel(x, past_k, past_v, W_qkv, W_proj):
    out, _ = run(x, past_k, past_v, W_qkv, W_proj, trace=False)
    return out
